# revision 1
# baseline (speedup 1.0000x reference)
"""nn_Decoder kernel: 12-step goal/action LSTM decoder + per-scene 2-layer GAT.

Strategy (per sharding hint): data-parallel over scenes — shard the pedestrian
axis (B=32768, 512 uniform scenes of 64) across the 8 NeuronCores; all weights
are tiny and replicated; the per-timestep scan stays local per shard since GAT
attention never crosses scene boundaries.

Self-contained: shapes hardcoded, no sibling imports.
"""
import numpy as np

OBS_LEN, PRED_LEN = 8, 12
B, NG, G = 32768, 512, 64
GH, AH = 64, 32
NEG_SLOPE = 0.2
NCORES = 8


def _np_f(x):
    return np.asarray(x, np.float32)


def _run_sharded_jax(args):
    """Run the decoder on 8 devices, batch sharded; returns (pred_goal, pred_action)."""
    import jax
    import jax.numpy as jnp

    devs = jax.devices()
    if len(devs) < NCORES:
        raise RuntimeError(f"need {NCORES} devices, have {len(devs)}")

    (gh0, ah0, xg0, xa0, Wih_g, Whh_g, bg, Wih_a, Whh_a, ba,
     W_h2g, b_h2g, W_h2a, b_h2a, W_ga, b_ga, W_aa, b_aa,
     w0, asrc0, adst0, bias0, w1, asrc1, adst1, bias1) = [jnp.asarray(a) for a in args]

    bs = B // NCORES          # 4096 peds per core
    ng = NG // NCORES         # 64 scenes per core

    def lstm(x, h, c, Wih, Whh, b):
        z = x @ Wih.T + h @ Whh.T + b
        i, f, g, o = jnp.split(z, 4, axis=-1)
        c = jax.nn.sigmoid(f) * c + jax.nn.sigmoid(i) * jnp.tanh(g)
        return jax.nn.sigmoid(o) * jnp.tanh(c), c

    def inorm(x):
        m = x.mean(axis=1, keepdims=True)
        v = x.var(axis=1, keepdims=True)
        return (x - m) * jax.lax.rsqrt(v + 1e-5)

    def gat_layer(x, w, a_src, a_dst, bb):
        hp = jnp.einsum('gnf,hfo->ghno', x, w)
        s = jnp.einsum('ghno,hoi->ghni', hp, a_src)
        d = jnp.einsum('ghno,hoi->ghni', hp, a_dst)
        attn = jax.nn.softmax(
            jax.nn.leaky_relu(s + d.swapaxes(-1, -2), NEG_SLOPE), axis=-1)
        return jnp.einsum('ghnm,ghmo->ghno', attn, hp) + bb

    def gat(x):
        y = x.reshape(ng, G, AH)
        y = gat_layer(inorm(y), w0, asrc0, adst0, bias0)
        y = jax.nn.elu(y.transpose(0, 2, 1, 3).reshape(ng, G, -1))
        y = gat_layer(inorm(y), w1, asrc1, adst1, bias1)[:, 0]
        return y.reshape(bs, AH)

    def shard_fn(gh_s, ah_s, xg_s, xa_s):
        def step(carry, _):
            ghh, gcc, ahh, acc, go, ao = carry
            ghh, gcc = lstm(go, ghh, gcc, Wih_g, Whh_g, bg)
            go = ghh @ W_h2g.T + b_h2g
            ahh, acc = lstm(ao, ahh, acc, Wih_a, Whh_a, ba)
            ahh = ahh * jax.nn.softmax(go @ W_ga.T + b_ga, axis=-1)
            ahh = gat(ahh)
            ao = ahh @ W_h2a.T + b_h2a
            ghh = ghh * jax.nn.softmax(ao @ W_aa.T + b_aa, axis=-1)
            return (ghh, gcc, ahh, acc, go, ao), (go, ao)

        init = (gh_s, jnp.zeros_like(gh_s), ah_s, jnp.zeros_like(ah_s), xg_s, xa_s)
        _, (pg, pa) = jax.lax.scan(step, init, None, length=PRED_LEN)
        return pg, pa

    pmapped = jax.pmap(shard_fn, axis_name="x")
    gh_sh = gh0.reshape(NCORES, bs, GH)
    ah_sh = ah0.reshape(NCORES, bs, AH)
    xg_sh = xg0.reshape(NCORES, bs, 2)
    xa_sh = xa0.reshape(NCORES, bs, 2)
    pg, pa = pmapped(gh_sh, ah_sh, xg_sh, xa_sh)   # [8, 12, bs, 2]
    pg = np.asarray(pg).transpose(1, 0, 2, 3).reshape(PRED_LEN, B, 2)
    pa = np.asarray(pa).transpose(1, 0, 2, 3).reshape(PRED_LEN, B, 2)
    return np.asarray(pg, np.float32), np.asarray(pa, np.float32)


def _run_numpy(args):
    """Vectorized numpy fallback (validated vs reference to ~2e-5 rel)."""
    (gh, ah, xg0, xa0, Wih_g, Whh_g, bg, Wih_a, Whh_a, ba,
     W_h2g, b_h2g, W_h2a, b_h2a, W_ga, b_ga, W_aa, b_aa,
     w0, asrc0, adst0, bias0, w1, asrc1, adst1, bias1) = args

    def sigmoid(x):
        return 1.0 / (1.0 + np.exp(-x))

    def cell(z, c, H):
        i, fg, g, o = z[:, :H], z[:, H:2*H], z[:, 2*H:3*H], z[:, 3*H:]
        c = sigmoid(fg) * c + sigmoid(i) * np.tanh(g)
        return sigmoid(o) * np.tanh(c), c

    def softmax(x):
        e = np.exp(x - x.max(-1, keepdims=True))
        return e / e.sum(-1, keepdims=True)

    def inorm(x):
        m = x.mean(1, keepdims=True)
        v = x.var(1, keepdims=True)
        return (x - m) / np.sqrt(v + 1e-5)

    def gat_layer(x, wcat, ws, wd, bias, nh, fo):
        hp = x @ wcat                       # [ng, 64, nh*fo]
        s = x @ ws                          # [ng, 64, nh]
        d = x @ wd
        outs = []
        for h in range(nh):
            pre = s[:, :, h:h+1] + d[:, None, :, h]
            e = np.exp(np.maximum(pre, NEG_SLOPE * pre) - 0.0)
            num = e @ hp[:, :, h*fo:(h+1)*fo]
            den = e.sum(-1, keepdims=True)
            outs.append(num / den)
        return np.concatenate(outs, -1) + np.tile(bias, nh)

    w0cat = w0.transpose(1, 0, 2).reshape(32, 64)
    ws0 = np.concatenate([w0[h] @ asrc0[h] for h in range(4)], 1)
    wd0 = np.concatenate([w0[h] @ adst0[h] for h in range(4)], 1)
    w1cat, ws1, wd1 = w1[0], w1[0] @ asrc1[0], w1[0] @ adst1[0]

    gc = np.zeros_like(gh)
    ac = np.zeros_like(ah)
    go, ao = xg0, xa0
    pgs, pas = [], []
    for _ in range(PRED_LEN):
        zg = go @ Wih_g.T + gh @ Whh_g.T + bg
        gh_pc, gc = cell(zg, gc, GH)
        go = gh_pc @ W_h2g.T + b_h2g
        pgs.append(go)
        za = ao @ Wih_a.T + ah @ Whh_a.T + ba
        ah_l, ac = cell(za, ac, AH)
        ah_l = ah_l * softmax(go @ W_ga.T + b_ga)
        x = inorm(ah_l.reshape(NG, G, AH))
        y = gat_layer(x, w0cat, ws0, wd0, bias0, 4, 16)
        y = np.where(y > 0, y, np.exp(np.minimum(y, 0.0)) - 1.0)
        y = gat_layer(inorm(y), w1cat, ws1, wd1, bias1, 1, 32)
        ah = y.reshape(B, AH)
        pas.append(ah @ W_h2a.T + b_h2a)
        ao = pas[-1]
        gh = gh_pc * softmax(ao @ W_aa.T + b_aa)
    return (np.stack(pgs).astype(np.float32), np.stack(pas).astype(np.float32))


def kernel(teacher_forcing_ratio, seq_start_end, goal_real, goal_input_hidden_state,
           action_real, action_input_hidden_state,
           Wih_g, Whh_g, bih_g, bhh_g, W_h2g, b_h2g,
           Wih_a, Whh_a, bih_a, bhh_a, W_h2a, b_h2a,
           W_ga, b_ga, W_aa, b_aa,
           w0, asrc0, adst0, bias0, w1, asrc1, adst1, bias1):
    args = (
        _np_f(goal_input_hidden_state), _np_f(action_input_hidden_state),
        _np_f(goal_real)[OBS_LEN - 1], _np_f(action_real)[OBS_LEN - 1],
        _np_f(Wih_g), _np_f(Whh_g), _np_f(bih_g) + _np_f(bhh_g),
        _np_f(Wih_a), _np_f(Whh_a), _np_f(bih_a) + _np_f(bhh_a),
        _np_f(W_h2g), _np_f(b_h2g), _np_f(W_h2a), _np_f(b_h2a),
        _np_f(W_ga), _np_f(b_ga), _np_f(W_aa), _np_f(b_aa),
        _np_f(w0), _np_f(asrc0), _np_f(adst0), _np_f(bias0),
        _np_f(w1), _np_f(asrc1), _np_f(adst1), _np_f(bias1),
    )
    # jax biases: pass raw biases separately for the jax path (it uses both)
    jargs = (args[0], args[1], args[2], args[3],
             args[4], args[5], args[6], args[7], args[8], args[9],
             args[10], args[11], args[12], args[13], args[14], args[15],
             args[16], args[17], args[18], args[19], args[20], args[21],
             args[22], args[23], args[24], args[25])
    try:
        return _run_sharded_jax(jargs)
    except Exception:
        return _run_numpy(args)


# revision 2
# speedup vs baseline: 182.4332x; 182.4332x over previous
"""nn_Decoder kernel: 12-step goal/action LSTM decoder + per-scene 2-layer GAT.

Strategy (per sharding hint): data-parallel over scenes — shard the pedestrian
axis (B=32768, 512 uniform scenes of 64) across the 8 NeuronCores; all weights
are tiny and replicated; the per-timestep scan stays local per shard since GAT
attention never crosses scene boundaries.

Self-contained: shapes hardcoded, no sibling imports.
"""
import numpy as np

OBS_LEN, PRED_LEN = 8, 12
B, NG, G = 32768, 512, 64
GH, AH = 64, 32
NEG_SLOPE = 0.2
NCORES = 8


def _np_f(x):
    return np.asarray(x, np.float32)


_PMAP_CACHE = {}


def _get_pmapped():
    """Build (once per process) the pmapped shard function. Weights are passed
    as broadcast (in_axes=None) args so repeat calls reuse the compiled NEFF."""
    if "fn" in _PMAP_CACHE:
        return _PMAP_CACHE["fn"]
    import jax
    import jax.numpy as jnp

    devs = jax.devices()
    if len(devs) < NCORES:
        raise RuntimeError(f"need {NCORES} devices, have {len(devs)}")

    bs = B // NCORES          # 4096 peds per core
    ng = NG // NCORES         # 64 scenes per core

    def lstm(x, h, c, Wih, Whh, b):
        z = x @ Wih.T + h @ Whh.T + b
        i, f, g, o = jnp.split(z, 4, axis=-1)
        c = jax.nn.sigmoid(f) * c + jax.nn.sigmoid(i) * jnp.tanh(g)
        return jax.nn.sigmoid(o) * jnp.tanh(c), c

    def inorm(x):
        m = x.mean(axis=1, keepdims=True)
        v = x.var(axis=1, keepdims=True)
        return (x - m) * jax.lax.rsqrt(v + 1e-5)

    def gat_layer(x, w, a_src, a_dst, bb):
        hp = jnp.einsum('gnf,hfo->ghno', x, w)
        s = jnp.einsum('ghno,hoi->ghni', hp, a_src)
        d = jnp.einsum('ghno,hoi->ghni', hp, a_dst)
        attn = jax.nn.softmax(
            jax.nn.leaky_relu(s + d.swapaxes(-1, -2), NEG_SLOPE), axis=-1)
        return jnp.einsum('ghnm,ghmo->ghno', attn, hp) + bb

    def shard_fn(gh_s, ah_s, xg_s, xa_s, W):
        def gat(x):
            y = x.reshape(ng, G, AH)
            y = gat_layer(inorm(y), W["w0"], W["asrc0"], W["adst0"], W["bias0"])
            y = jax.nn.elu(y.transpose(0, 2, 1, 3).reshape(ng, G, -1))
            y = gat_layer(inorm(y), W["w1"], W["asrc1"], W["adst1"], W["bias1"])[:, 0]
            return y.reshape(bs, AH)

        def step(carry, _):
            ghh, gcc, ahh, acc, go, ao = carry
            ghh, gcc = lstm(go, ghh, gcc, W["Wih_g"], W["Whh_g"], W["bg"])
            go = ghh @ W["W_h2g"].T + W["b_h2g"]
            ahh, acc = lstm(ao, ahh, acc, W["Wih_a"], W["Whh_a"], W["ba"])
            ahh = ahh * jax.nn.softmax(go @ W["W_ga"].T + W["b_ga"], axis=-1)
            ahh = gat(ahh)
            ao = ahh @ W["W_h2a"].T + W["b_h2a"]
            ghh = ghh * jax.nn.softmax(ao @ W["W_aa"].T + W["b_aa"], axis=-1)
            return (ghh, gcc, ahh, acc, go, ao), (go, ao)

        init = (gh_s, jnp.zeros_like(gh_s), ah_s, jnp.zeros_like(ah_s), xg_s, xa_s)
        _, (pg, pa) = jax.lax.scan(step, init, None, length=PRED_LEN)
        return pg, pa

    fn = jax.pmap(shard_fn, axis_name="x", in_axes=(0, 0, 0, 0, None))
    _PMAP_CACHE["fn"] = fn
    return fn


def _run_sharded_jax(args):
    """Run the decoder on 8 devices, batch sharded; returns (pred_goal, pred_action)."""
    (gh0, ah0, xg0, xa0, Wih_g, Whh_g, bg, Wih_a, Whh_a, ba,
     W_h2g, b_h2g, W_h2a, b_h2a, W_ga, b_ga, W_aa, b_aa,
     w0, asrc0, adst0, bias0, w1, asrc1, adst1, bias1) = args
    fn = _get_pmapped()
    bs = B // NCORES
    W = dict(Wih_g=Wih_g, Whh_g=Whh_g, bg=bg, Wih_a=Wih_a, Whh_a=Whh_a, ba=ba,
             W_h2g=W_h2g, b_h2g=b_h2g, W_h2a=W_h2a, b_h2a=b_h2a,
             W_ga=W_ga, b_ga=b_ga, W_aa=W_aa, b_aa=b_aa,
             w0=w0, asrc0=asrc0, adst0=adst0, bias0=bias0,
             w1=w1, asrc1=asrc1, adst1=adst1, bias1=bias1)
    pg, pa = fn(gh0.reshape(NCORES, bs, GH), ah0.reshape(NCORES, bs, AH),
                xg0.reshape(NCORES, bs, 2), xa0.reshape(NCORES, bs, 2), W)
    pg = np.asarray(pg).transpose(1, 0, 2, 3).reshape(PRED_LEN, B, 2)
    pa = np.asarray(pa).transpose(1, 0, 2, 3).reshape(PRED_LEN, B, 2)
    return np.asarray(pg, np.float32), np.asarray(pa, np.float32)


def _run_numpy(args):
    """Vectorized numpy fallback (validated vs reference to ~2e-5 rel)."""
    (gh, ah, xg0, xa0, Wih_g, Whh_g, bg, Wih_a, Whh_a, ba,
     W_h2g, b_h2g, W_h2a, b_h2a, W_ga, b_ga, W_aa, b_aa,
     w0, asrc0, adst0, bias0, w1, asrc1, adst1, bias1) = args

    def sigmoid(x):
        return 1.0 / (1.0 + np.exp(-x))

    def cell(z, c, H):
        i, fg, g, o = z[:, :H], z[:, H:2*H], z[:, 2*H:3*H], z[:, 3*H:]
        c = sigmoid(fg) * c + sigmoid(i) * np.tanh(g)
        return sigmoid(o) * np.tanh(c), c

    def softmax(x):
        e = np.exp(x - x.max(-1, keepdims=True))
        return e / e.sum(-1, keepdims=True)

    def inorm(x):
        m = x.mean(1, keepdims=True)
        v = x.var(1, keepdims=True)
        return (x - m) / np.sqrt(v + 1e-5)

    def gat_layer(x, wcat, ws, wd, bias, nh, fo):
        hp = x @ wcat                       # [ng, 64, nh*fo]
        s = x @ ws                          # [ng, 64, nh]
        d = x @ wd
        outs = []
        for h in range(nh):
            pre = s[:, :, h:h+1] + d[:, None, :, h]
            e = np.exp(np.maximum(pre, NEG_SLOPE * pre) - 0.0)
            num = e @ hp[:, :, h*fo:(h+1)*fo]
            den = e.sum(-1, keepdims=True)
            outs.append(num / den)
        return np.concatenate(outs, -1) + np.tile(bias, nh)

    w0cat = w0.transpose(1, 0, 2).reshape(32, 64)
    ws0 = np.concatenate([w0[h] @ asrc0[h] for h in range(4)], 1)
    wd0 = np.concatenate([w0[h] @ adst0[h] for h in range(4)], 1)
    w1cat, ws1, wd1 = w1[0], w1[0] @ asrc1[0], w1[0] @ adst1[0]

    gc = np.zeros_like(gh)
    ac = np.zeros_like(ah)
    go, ao = xg0, xa0
    pgs, pas = [], []
    for _ in range(PRED_LEN):
        zg = go @ Wih_g.T + gh @ Whh_g.T + bg
        gh_pc, gc = cell(zg, gc, GH)
        go = gh_pc @ W_h2g.T + b_h2g
        pgs.append(go)
        za = ao @ Wih_a.T + ah @ Whh_a.T + ba
        ah_l, ac = cell(za, ac, AH)
        ah_l = ah_l * softmax(go @ W_ga.T + b_ga)
        x = inorm(ah_l.reshape(NG, G, AH))
        y = gat_layer(x, w0cat, ws0, wd0, bias0, 4, 16)
        y = np.where(y > 0, y, np.exp(np.minimum(y, 0.0)) - 1.0)
        y = gat_layer(inorm(y), w1cat, ws1, wd1, bias1, 1, 32)
        ah = y.reshape(B, AH)
        pas.append(ah @ W_h2a.T + b_h2a)
        ao = pas[-1]
        gh = gh_pc * softmax(ao @ W_aa.T + b_aa)
    return (np.stack(pgs).astype(np.float32), np.stack(pas).astype(np.float32))


def kernel(teacher_forcing_ratio, seq_start_end, goal_real, goal_input_hidden_state,
           action_real, action_input_hidden_state,
           Wih_g, Whh_g, bih_g, bhh_g, W_h2g, b_h2g,
           Wih_a, Whh_a, bih_a, bhh_a, W_h2a, b_h2a,
           W_ga, b_ga, W_aa, b_aa,
           w0, asrc0, adst0, bias0, w1, asrc1, adst1, bias1):
    args = (
        _np_f(goal_input_hidden_state), _np_f(action_input_hidden_state),
        _np_f(goal_real)[OBS_LEN - 1], _np_f(action_real)[OBS_LEN - 1],
        _np_f(Wih_g), _np_f(Whh_g), _np_f(bih_g) + _np_f(bhh_g),
        _np_f(Wih_a), _np_f(Whh_a), _np_f(bih_a) + _np_f(bhh_a),
        _np_f(W_h2g), _np_f(b_h2g), _np_f(W_h2a), _np_f(b_h2a),
        _np_f(W_ga), _np_f(b_ga), _np_f(W_aa), _np_f(b_aa),
        _np_f(w0), _np_f(asrc0), _np_f(adst0), _np_f(bias0),
        _np_f(w1), _np_f(asrc1), _np_f(adst1), _np_f(bias1),
    )
    # jax biases: pass raw biases separately for the jax path (it uses both)
    jargs = (args[0], args[1], args[2], args[3],
             args[4], args[5], args[6], args[7], args[8], args[9],
             args[10], args[11], args[12], args[13], args[14], args[15],
             args[16], args[17], args[18], args[19], args[20], args[21],
             args[22], args[23], args[24], args[25])
    try:
        return _run_sharded_jax(jargs)
    except Exception:
        return _run_numpy(args)
